# revision 13
# baseline (speedup 1.0000x reference)
"""Trainium2 Bass kernel for the hardest-positive triplet-softplus loss.

Strategy v2 (label-sorted windows, 8 NeuronCores):
  - Host sorts rows AND columns of the distance matrix by label (a pure
    layout/sharding choice).  After the sort, the same-label columns a
    128-row tile needs all live in one narrow window (max 183 cols for
    this input regime) -> pad to W (multiple of 64).  The dense 4096-col
    scan of the baseline becomes a W-col scan: ~16x less PE+DVE work.
  - Per 128-row tile the PE computes one [128, W+128] fp16 matmul group:
    cols [0,W) are the window (selection), cols [W,W+128) are the rows'
    negatives, so d(a,n) comes from the same matmul (diagonal extract).
  - Selection: ttr fuses (2dot + adj) with a row-max; adj carries
    -fp16(sq_j-512), the not-my-label -BIG mask and the self -2BIG mask.
    A gpsimd stt (is_equal * g) then recovers g_p = (sq_p-512)-fp16(sq_p-512)
    of the winning column, so d2ap = sq_a + g_p + 512 - rowmax is exact
    up to the fp16 rounding of the dot itself (~0.02 in d2 ~ 900).
  - d2an = sq_a + sq_n - diag(2dot(a,n)), zeroed for self-negatives.
  - Tail: d*10 via one Sqrt activation (scale=100), stable softplus via
    relu/abs_max on DVE + Exp/Ln on scalar, valid-mask, row-reduce.
  - Each core returns a [128,1] partial; host sums and divides by the
    host-known valid count (the unshard step).
"""

import os
import sys

import numpy as np

for _p in ("/opt/trn_rl_repo", "/root/.axon_site/_ro/trn_rl_repo"):
    if os.path.isdir(_p) and _p not in sys.path:
        sys.path.append(_p)

import concourse.bass as bass  # noqa: E402,F401
import concourse.bacc as bacc  # noqa: E402
import concourse.tile as tile  # noqa: E402
from concourse import mybir  # noqa: E402
from concourse import bass_utils  # noqa: E402

B = 4096
DIM = 512
C = 128
NCORES = 8
RB = B // NCORES          # rows per core
NT = RB // 128            # row tiles per core
NK = DIM // 128           # k chunks
BIG = 4096.0
EPS = 1e-8

F32 = mybir.dt.float32
F16 = mybir.dt.float16
ALU = mybir.AluOpType
AFT = mybir.ActivationFunctionType
AX = mybir.AxisListType

_NC_CACHE = {}


def _build_nc(W):
    WP = W + 128              # window + negatives block
    BLOBW = NK * WP + 2 * W   # rhs chunks | adj | g16

    nc = bacc.Bacc(
        "TRN2",
        target_bir_lowering=False,
        debug=False,
        enable_asserts=False,
    )

    lhs_d = nc.dram_tensor("lhsd", [NT, 128, NK * 128], F16,
                           kind="ExternalInput").ap()
    blob_d = nc.dram_tensor("blob", [NT, 128, BLOBW], F16, kind="ExternalInput").ap()
    diag_d = nc.dram_tensor("diagm", [128, 128], F16, kind="ExternalInput").ap()
    small_d = nc.dram_tensor("small", [128, 16], F32, kind="ExternalInput").ap()
    out_d = nc.dram_tensor("out", [128, 1], F32, kind="ExternalOutput").ap()

    with tile.TileContext(nc) as tc:
        with (
            tc.tile_pool(name="cst", bufs=1) as cst,
            tc.tile_pool(name="work", bufs=2) as work,
            tc.tile_pool(name="ps", bufs=4, space="PSUM") as pp,
            tc.tile_pool(name="sm", bufs=1) as sm,
        ):
            # per-tile lhs/blob DMAs interleaved so tile 0 lands first
            lhs_sb, blob_sb = [], []
            for t in range(NT):
                lt = cst.tile([128, NK * 128], F16, tag=f"lhs{t}",
                              name=f"lhs{t}")
                nc.sync.dma_start(lt[:], lhs_d[t])
                lhs_sb.append(lt)
                bt = cst.tile([128, BLOBW], F16, tag=f"blob{t}", name=f"blob{t}")
                nc.sync.dma_start(bt[:], blob_d[t])
                blob_sb.append(bt)
            diag_sb = cst.tile([128, 128], F16, tag="diag")
            nc.sync.dma_start(diag_sb[:], diag_d[:])
            small_sb = cst.tile([128, 16], F32, tag="small")
            nc.sync.dma_start(small_sb[:], small_d[:])

            epsb = sm.tile([128, 1], F32, tag="epsb")
            nc.gpsimd.memset(epsb[:], EPS)
            # dummies: pull all three act table loads into the DMA phase
            dumm = sm.tile([128, 1], F32, tag="dumm")
            nc.scalar.activation(dumm[:], epsb[:], AFT.Sqrt)
            dumm2 = sm.tile([128, 1], F32, tag="dumm2")
            nc.scalar.activation(dumm2[:], epsb[:], AFT.Exp)
            dumm3 = sm.tile([128, 1], F32, tag="dumm3")
            nc.scalar.activation(dumm3[:], epsb[:], AFT.Ln, bias=1.0)

            rowmaxv = sm.tile([128, NT], F32, tag="rowmaxv")
            gsel = sm.tile([128, NT], F32, tag="gsel")
            dotan = sm.tile([128, NT], F32, tag="dotan")

            diag_ap = diag_sb[:]

            for t in range(NT):
                P = pp.tile([128, WP], F32, tag="acc", name=f"acc{t}")
                for kk in range(NK):
                    nc.tensor.matmul(
                        P[:],
                        lhs_sb[t][:, kk * 128:(kk + 1) * 128],
                        blob_sb[t][:, kk * WP:(kk + 1) * WP],
                        start=(kk == 0),
                        stop=(kk == NK - 1),
                    )
                # d(a,n): extract diagonal of the negatives block
                junk3 = work.tile([128, 128], F32, tag="junk3", name="junk3")
                nc.vector.scalar_tensor_tensor(
                    junk3[:], P[:, W:WP], 1.0, diag_ap,
                    op0=ALU.mult, op1=ALU.mult,
                    accum_out=dotan[:, t:t + 1],
                )
                # selection: masked S = 2dot + adj, row max
                junk = work.tile([128, W], F32, tag="junk", name="junk")
                nc.vector.tensor_add(
                    junk[:], P[:, 0:W],
                    blob_sb[t][:, NK * WP:NK * WP + W],
                )
                nc.vector.tensor_reduce(
                    rowmaxv[:, t:t + 1], junk[:], axis=AX.X, op=ALU.max,
                )
                # winner's g residual via (S == rowmax) * g
                junk2 = work.tile([128, W], F32, tag="junk2", name="junk2")
                nc.vector.scalar_tensor_tensor(
                    junk2[:], junk[:], rowmaxv[:, t:t + 1],
                    blob_sb[t][:, NK * WP + W:NK * WP + 2 * W],
                    op0=ALU.is_equal, op1=ALU.mult,
                    accum_out=gsel[:, t:t + 1],
                )

            sqa = small_sb[:, 0:4]
            sqn = small_sb[:, 4:8]
            nmask = small_sb[:, 8:12]
            vldv = small_sb[:, 12:16]

            pack = sm.tile([128, 2 * NT], F32, tag="pack")
            t1 = sm.tile([128, NT], F32, tag="t1")
            nc.vector.tensor_add(t1[:], sqa, gsel[:])
            t2 = sm.tile([128, NT], F32, tag="t2")
            nc.vector.tensor_scalar(t2[:], t1[:], 512.0, None, op0=ALU.add)
            nc.vector.tensor_sub(pack[:, 0:NT], t2[:], rowmaxv[:])
            u1 = sm.tile([128, NT], F32, tag="u1")
            nc.vector.tensor_add(u1[:], sqa, sqn)
            u2 = sm.tile([128, NT], F32, tag="u2")
            nc.vector.tensor_sub(u2[:], u1[:], dotan[:])
            nc.vector.tensor_mul(pack[:, NT:2 * NT], u2[:], nmask)
            nc.vector.tensor_scalar(pack[:], pack[:], 0.0, None, op0=ALU.max)

            d10 = sm.tile([128, 2 * NT], F32, tag="d10")
            nc.scalar.activation(d10[:], pack[:], AFT.Sqrt,
                                 bias=epsb[:], scale=100.0)
            z = sm.tile([128, NT], F32, tag="z")
            nc.vector.tensor_sub(z[:], d10[:, 0:NT], d10[:, NT:2 * NT])
            a1 = sm.tile([128, NT], F32, tag="a1")
            nc.vector.tensor_scalar(a1[:], z[:], 0.0, None, op0=ALU.max)
            az = sm.tile([128, NT], F32, tag="az")
            nc.scalar.activation(az[:], z[:], AFT.Abs)
            e = sm.tile([128, NT], F32, tag="e")
            nc.scalar.activation(e[:], az[:], AFT.Exp, scale=-1.0)
            ln1p = sm.tile([128, NT], F32, tag="ln1p")
            nc.scalar.activation(ln1p[:], e[:], AFT.Ln, bias=1.0)
            per = sm.tile([128, NT], F32, tag="per")
            nc.vector.tensor_add(per[:], a1[:], ln1p[:])
            wv = sm.tile([128, NT], F32, tag="wv")
            nc.vector.tensor_mul(wv[:], per[:], vldv)
            prt = sm.tile([128, 1], F32, tag="prt")
            nc.vector.tensor_reduce(prt[:], wv[:], axis=AX.X, op=ALU.add)
            nc.sync.dma_start(out_d[:], prt[:])

    nc.compile()
    return nc


def get_nc(W=192):
    if W not in _NC_CACHE:
        _NC_CACHE[W] = _build_nc(W)
    return _NC_CACHE[W]


def _prep_inputs(batch, labels, anchors, negatives):
    """Host-side sharding prep: label-sort layout + per-core input maps."""
    batch = np.ascontiguousarray(np.asarray(batch), dtype=np.float32)
    labels = np.asarray(labels).astype(np.int64)
    anchors = np.asarray(anchors).astype(np.int64)
    negatives = np.asarray(negatives).astype(np.int64)
    assert np.array_equal(anchors, np.arange(B)), "kernel assumes anchors=arange"

    sq = np.einsum("ij,ij->i", batch, batch, dtype=np.float32,
                   optimize=True).astype(np.float32)

    perm = np.argsort(labels, kind="stable")
    ls = labels[perm]                       # sorted labels
    xs16 = batch[perm].astype(np.float16)   # sorted rows, fp16
    lhs16 = (xs16 * np.float16(2.0))        # exact scale
    sqs = sq[perm]
    sqn512 = sqs - np.float32(512.0)
    sqf16 = sqn512.astype(np.float16)
    sqf16_32 = sqf16.astype(np.float32)
    g16 = (sqn512 - sqf16_32).astype(np.float16)

    col_start = np.searchsorted(ls, np.arange(C), side="left")
    col_end = np.searchsorted(ls, np.arange(C), side="right")

    NTILES = B // 128
    w0 = np.empty(NTILES, np.int64)
    need = 0
    for T in range(NTILES):
        w0[T] = col_start[ls[T * 128]]
        need = max(need, col_end[ls[T * 128 + 127]] - w0[T])
    W = max(64, int(-(-need // 64) * 64))
    assert W <= 384, f"window {need} too wide"
    WP = W + 128
    BLOBW = NK * WP + 2 * W

    negs_s = negatives[perm]
    xneg16 = batch[negs_s].astype(np.float16)
    sq_neg = sq[negs_s]
    nmask = (negs_s != perm).astype(np.float32)
    hist = np.bincount(labels, minlength=C)
    vld_all = (hist[labels] >= 3).astype(np.float32)[perm]
    count = float(vld_all.sum())

    qs = np.arange(W)
    ms = np.arange(128)
    in_maps = []
    for c in range(NCORES):
        blob = np.empty((NT, 128, BLOBW), np.float16)
        lhsd = np.empty((NT, 128, NK * 128), np.float16)
        smalls = np.empty((128, 16), np.float32)
        for t in range(NT):
            T = c * NT + t
            rows = slice(T * 128, (T + 1) * 128)
            wcols = (w0[T] + qs) % B
            rhs_k = np.concatenate(
                [xs16[wcols].T, xneg16[rows].T], axis=1)      # [512, WP]
            blob[t, :, :NK * WP] = rhs_k.reshape(
                NK, 128, WP).transpose(1, 0, 2).reshape(128, NK * WP)
            lm = ls[rows]
            lq = ls[wcols]
            adj = np.repeat(-sqf16_32[wcols][None, :], 128, axis=0)
            adj[lq[None, :] != lm[:, None]] -= BIG
            q_self = T * 128 + ms - w0[T]
            assert (q_self >= 0).all() and (q_self < W).all()
            adj[ms, q_self] -= 2.0 * BIG
            blob[t, :, NK * WP:NK * WP + W] = adj.astype(np.float16)
            blob[t, :, NK * WP + W:] = np.repeat(
                g16[wcols][None, :], 128, axis=0)

            lrows = lhs16[rows]                                # [128m, 512k]
            lhsd[t] = lrows.T.reshape(NK, 128, 128).transpose(1, 0, 2).reshape(
                128, NK * 128)
            smalls[:, t] = sqs[rows]
            smalls[:, 4 + t] = sq_neg[rows]
            smalls[:, 8 + t] = nmask[rows]
            smalls[:, 12 + t] = vld_all[rows]
        in_maps.append({
            "lhsd": np.ascontiguousarray(lhsd),
            "blob": np.ascontiguousarray(blob),
            "diagm": np.eye(128, dtype=np.float16),
            "small": np.ascontiguousarray(smalls),
        })
    return in_maps, count, W


def kernel(batch, labels, anchors, negatives, **_kwargs):
    in_maps, count, W = _prep_inputs(batch, labels, anchors, negatives)
    nc = get_nc(W)
    res = bass_utils.run_bass_kernel_spmd(nc, in_maps, core_ids=list(range(NCORES)))
    total = sum(r["out"].sum(dtype=np.float64) for r in res.results)
    loss = np.float32(np.float32(total) / np.float32(count))
    return np.array([loss], dtype=np.float32)
